# revision 2
# baseline (speedup 1.0000x reference)
"""Trainium2 Bass kernel for nn_LinkPredictor (MoE-routed bilinear link scorer).

score[b] = head[b]^T @ W[rel_id[b]] @ tail[b] + sum(b[rel_id[b]])

Strategy (relation sharding / MoE routing on host, dense matmuls on device):
  * Host groups samples by relation (argsort of rel_id), splits each
    relation's samples into slots of <=128, and assigns slots to the 8
    NeuronCores balanced by sample count.  Each core sees S slots.
  * Per slot the device computes Q = H_slot @ W[r]  via 4 PE matmuls
    (contraction over e in chunks of 128; stationary = transposed heads
    [e_chunk, samples], moving = W[r][e_chunk, :] streamed at N=512),
    accumulated in one PSUM bank as Q[sample, d].
  * One fused DVE op per slot: score = reduce_add(Q * tail, axis=free)
    with the per-relation bias sum as the reduction's initial value.
  * W is only ever read once from HBM across the whole machine
    (32 MB total -> 4 MB per core), which is the bandwidth floor.
"""

import os
import sys
import math

import numpy as np

for _p in ("/opt/trn_rl_repo",):
    if _p not in sys.path:
        sys.path.append(_p)

import concourse.bass as bass  # noqa: E402
import concourse.mybir as mybir  # noqa: E402
import concourse.tile as tile  # noqa: E402
from concourse import bacc  # noqa: E402
from concourse import bass_utils  # noqa: E402

B, D, R = 2048, 512, 32
N_CORES = 8
F32 = mybir.dt.float32


def _install_ntff_hook():
    """Provide antenv.axon_hooks if the image lacks it, so trace=True /
    BASS_TRACE=1 profiling works under axon (see trn_agent_boot.trn_boot)."""
    try:
        from antenv.axon_hooks import get_axon_ntff_profile_hook  # noqa: F401
        return
    except ImportError:
        pass
    import types
    try:
        import antenv
        from trn_agent_boot.trn_boot import _ntff_profile_via_ctypes
    except Exception:
        return
    mod = types.ModuleType("antenv.axon_hooks")
    _state = {"hook": None}
    try:
        _state["hook"] = _ntff_profile_via_ctypes("/opt/axon/libaxon_pjrt.so")
    except Exception:
        _state["hook"] = None

    def set_axon_ntff_profile_hook(h):
        _state["hook"] = h

    def get_axon_ntff_profile_hook():
        return _state["hook"]

    mod.set_axon_ntff_profile_hook = set_axon_ntff_profile_hook
    mod.get_axon_ntff_profile_hook = get_axon_ntff_profile_hook
    sys.modules["antenv.axon_hooks"] = mod
    antenv.axon_hooks = mod


_install_ntff_hook()

_PROGRAM_CACHE = {}


def _build_program(S):
    """Bass program for one core: S slots x (<=128 samples, one relation)."""
    nc = bacc.Bacc("TRN2", target_bir_lowering=False, debug=False,
                   num_devices=N_CORES)

    ht = nc.dram_tensor("ht", [D, S * 128], F32, kind="ExternalInput")
    tl = nc.dram_tensor("tl", [S * 128, D], F32, kind="ExternalInput")
    wc = nc.dram_tensor("wc", [S * D, D], F32, kind="ExternalInput")
    bs = nc.dram_tensor("bs", [128, S], F32, kind="ExternalInput")
    out = nc.dram_tensor("out", [128, S], F32, kind="ExternalOutput")

    with tile.TileContext(nc) as tc:
        with (
            tc.tile_pool(name="const", bufs=1) as cpool,
            tc.tile_pool(name="wpool", bufs=2) as wpool,
            tc.tile_pool(name="tpool", bufs=2) as tpool,
            tc.tile_pool(name="spool", bufs=2) as spool,
            tc.tile_pool(name="psum", bufs=2, space="PSUM") as pspool,
        ):
            # Transposed heads, one tile per e-chunk: [128, S*128]
            ht_tiles = []
            for c in range(4):
                t = cpool.tile([128, S * 128], F32, tag=f"ht{c}")
                nc.sync.dma_start(t[:], ht.ap()[c * 128:(c + 1) * 128, :])
                ht_tiles.append(t)

            bias = cpool.tile([128, S], F32, tag="bias")
            nc.sync.dma_start(bias[:], bs.ap())

            score = cpool.tile([128, S], F32, tag="score")

            for j in range(S):
                # W[r_j] as [p=128, c=4 e-chunks, d=512] (1 MB DMA)
                w_t = wpool.tile([128, 4, D], F32, tag="w")
                w_src = wc.ap()[j * D:(j + 1) * D, :].rearrange(
                    "(c p) d -> p c d", p=128)
                nc.sync.dma_start(w_t[:], w_src)

                tail_t = tpool.tile([128, D], F32, tag="tail")
                nc.sync.dma_start(tail_t[:], tl.ap()[j * 128:(j + 1) * 128, :])

                # Q[sample, d] = sum_e head[sample, e] * W[r][e, d]
                P = pspool.tile([128, D], F32, tag="P")
                for c in range(4):
                    nc.tensor.matmul(
                        P[:],
                        ht_tiles[c][:, j * 128:(j + 1) * 128],
                        w_t[:, c, :],
                        start=(c == 0),
                        stop=(c == 3),
                    )

                # score[:, j] = sum_d(Q * tail)   (+ bias after the loop)
                # NOTE: fused tensor_tensor_reduce passes CoreSim but crashes
                # the exec unit on HW, so use separate mul + reduce.
                prod = spool.tile([128, D], F32, tag="prod")
                nc.vector.tensor_mul(prod[:], P[:], tail_t[:])
                nc.vector.tensor_reduce(
                    score[:, j:j + 1], prod[:],
                    axis=mybir.AxisListType.X, op=mybir.AluOpType.add,
                )

            final = cpool.tile([128, S], F32, tag="final")
            nc.vector.tensor_add(final[:], score[:], bias[:])
            nc.sync.dma_start(out.ap(), final[:])

    nc.compile()
    return nc


def _route(rel):
    """Group samples by relation into slots of <=128; balance across cores.

    Returns (S, core_slots) where core_slots[c] is a list of exactly S
    (relation, sample_indices) pairs (sample_indices possibly empty)."""
    counts = np.bincount(rel, minlength=R)
    order = np.argsort(rel, kind="stable")
    slots = []
    off = 0
    for r in range(R):
        n = int(counts[r])
        idx = order[off:off + n]
        off += n
        for c0 in range(0, n, 128):
            slots.append((r, idx[c0:c0 + 128]))
    S = max(1, math.ceil(len(slots) / N_CORES))
    # Greedy balance: biggest slots first onto least-loaded core with room.
    slots.sort(key=lambda s: -len(s[1]))
    core_slots = [[] for _ in range(N_CORES)]
    loads = [0] * N_CORES
    for r, idx in slots:
        cands = [c for c in range(N_CORES) if len(core_slots[c]) < S]
        c = min(cands, key=lambda c: loads[c])
        core_slots[c].append((r, idx))
        loads[c] += len(idx)
    empty = np.zeros(0, dtype=np.int64)
    for c in range(N_CORES):
        while len(core_slots[c]) < S:
            core_slots[c].append((0, empty))
    return S, core_slots


def kernel(head_emb, tail_emb, rel_id, W, b, **_unused):
    head_emb = np.ascontiguousarray(np.asarray(head_emb, dtype=np.float32))
    tail_emb = np.ascontiguousarray(np.asarray(tail_emb, dtype=np.float32))
    W = np.ascontiguousarray(np.asarray(W, dtype=np.float32))
    b = np.ascontiguousarray(np.asarray(b, dtype=np.float32))
    rel = np.asarray(rel_id).astype(np.int64)

    S, core_slots = _route(rel)

    if S not in _PROGRAM_CACHE:
        _PROGRAM_CACHE[S] = _build_program(S)
    nc = _PROGRAM_CACHE[S]

    bsum = b.astype(np.float64).sum(axis=1).astype(np.float32)

    in_maps = []
    for c in range(N_CORES):
        ht = np.zeros((D, S * 128), dtype=np.float32)
        tl = np.zeros((S * 128, D), dtype=np.float32)
        wc = np.empty((S * D, D), dtype=np.float32)
        bs = np.zeros((128, S), dtype=np.float32)
        for j, (r, idx) in enumerate(core_slots[c]):
            n = len(idx)
            if n:
                ht[:, j * 128:j * 128 + n] = head_emb[idx].T
                tl[j * 128:j * 128 + n, :] = tail_emb[idx]
            wc[j * D:(j + 1) * D, :] = W[r]
            bs[:, j] = bsum[r]
        in_maps.append({"ht": ht, "tl": tl, "wc": wc, "bs": bs})

    res = bass_utils.run_bass_kernel_spmd(nc, in_maps,
                                          core_ids=list(range(N_CORES)))

    scores = np.zeros(B, dtype=np.float32)
    for c in range(N_CORES):
        o = res.results[c]["out"]
        for j, (r, idx) in enumerate(core_slots[c]):
            n = len(idx)
            if n:
                scores[idx] = o[:n, j]
    return scores
